# revision 13
# baseline (speedup 1.0000x reference)
"""Trainium2 Bass kernel for CustomLSTMForecast.

B=512, T=256, I=256, H=512. Data-parallel: batch sharded 8 ways (64
rows/core), LSTM + fc weights replicated.

Per-core design (batch m = 64):
  Gate layout: each gate g lands in PSUM as [128, 256] — partitions
  0:64 = batch x hidden-lo (0:256), partitions 64:128 = batch x
  hidden-hi (256:512).  Written by N=256 matmuls column-tiled across
  two concurrent PE column groups (base partitions 0 / 64).  Two PSUM
  tiles per step:
     pFI [128, 512]: FD 0:256 = f-gate, 256:512 = i-gate
     pCO [128, 512]: FD 0:256 = c_hat,  256:512 = o-gate
  K-chunks: 4x h (hidden 512), 2x x (input 256), 1x ones row for the
  bias (one N=512 matmul per psum tile per column group).

  All elementwise runs at [128, 256] (full partition occupancy, half
  the free dim of the old [64, 512] layout) which nearly halves
  ACT/DVE time.  The h path is bf16 (tanh(c), sig(o), h) for 2x DVE
  mode; the c path stays f32.  h is transposed back to [hidden,
  batch] via 4 PE transposes + one DVE copy.

  Emission order shortens the serial chain: per column group the
  h-part matmuls run gate-major (f k0..k3, i, c_hat, o) so sigmoid(f)
  can start ~1.3us before the o-gate finishes; x-part matmuls of step
  t+1 are emitted before the elementwise of step t to keep PE busy.
"""
from contextlib import ExitStack

import numpy as np

import concourse.bass as bass
import concourse.tile as tile
from concourse import bacc, mybir
from concourse.bass_utils import run_bass_kernel_spmd

F32 = mybir.dt.float32
BF16 = mybir.dt.bfloat16
AF = mybir.ActivationFunctionType

B, T, I, H = 512, 256, 256, 512
NCORES = 8
BC = B // NCORES          # 64 batch rows per core
HH = H // 2               # 256: free size of a [128, 256] gate tile
KH = H // 128             # 4 hidden k-chunks
KX = I // 128             # 2 input k-chunks
NK = KH + KX + 1          # 7 k-chunks incl bias row

# gate chunk order in W_w: f=0, i=1, o=2, chat=3
# pFI holds (f, i) in FD halves; pCO holds (chat, o)
G_FI = (0, 1)
G_CO = (3, 2)

_CACHE = {}


def _build(nsteps=T):
    if nsteps in _CACHE:
        return _CACHE[nsteps]
    nc = bacc.Bacc("TRN2", target_bir_lowering=False, debug=False,
                   num_devices=NCORES)
    d_x = nc.dram_tensor("xT", [nsteps, 128, KX, BC], BF16,
                         kind="ExternalInput").ap()
    d_w = nc.dram_tensor("W", [NK, 128, 4, H], BF16,
                         kind="ExternalInput").ap()
    d_ones = nc.dram_tensor("ones_row", [128, BC], BF16,
                            kind="ExternalInput").ap()
    d_eye = nc.dram_tensor("eye", [128, BC], BF16, kind="ExternalInput").ap()
    d_fcw = nc.dram_tensor("fcw", [128, HH], F32, kind="ExternalInput").ap()
    d_fcb = nc.dram_tensor("fcb", [BC, 1], F32, kind="ExternalInput").ap()
    d_out = nc.dram_tensor("out", [BC, 1], F32, kind="ExternalOutput").ap()

    with tile.TileContext(nc) as tc, ExitStack() as ctx:
        _body(tc, ctx, nsteps, d_x, d_w, d_ones, d_eye, d_fcw, d_fcb, d_out)
    nc.compile()
    _CACHE[nsteps] = nc
    return nc


def _body(tc, ctx, nsteps, d_x, d_w, d_ones, d_eye, d_fcw, d_fcb, d_out):
    nc = tc.nc
    const = ctx.enter_context(tc.tile_pool(name="const", bufs=1))
    xpool = ctx.enter_context(tc.tile_pool(name="x", bufs=4))
    gact = ctx.enter_context(tc.tile_pool(name="gact", bufs=2))
    state = ctx.enter_context(tc.tile_pool(name="state", bufs=2))
    psFI = ctx.enter_context(tc.tile_pool(name="psFI", bufs=2, space="PSUM"))
    psCO = ctx.enter_context(tc.tile_pool(name="psCO", bufs=2, space="PSUM"))
    psT = ctx.enter_context(tc.tile_pool(name="psT", bufs=2, space="PSUM"))

    # W layout in SBUF: [128, NK, 4, H]; moving slice for (k-chunk j,
    # gate g, hidden half hh) is sW[:, j, g, 256*hh : 256*hh+256].
    sW = const.tile([128, NK, 4, H], BF16)
    nc.sync.dma_start(out=sW[:], in_=d_w.rearrange("k p g n -> p k g n"))
    s_ones = const.tile([128, BC], BF16)
    nc.sync.dma_start(out=s_ones[:], in_=d_ones)
    s_eye = const.tile([128, BC], BF16)
    nc.sync.dma_start(out=s_eye[:], in_=d_eye)
    s_fcw = const.tile([128, HH], F32)
    nc.sync.dma_start(out=s_fcw[:], in_=d_fcw)
    s_fcb = const.tile([BC, 1], F32)
    nc.sync.dma_start(out=s_fcb[:], in_=d_fcb)

    c_prev = state.tile([128, HH], F32, tag="c")
    nc.vector.memset(c_prev[:], 0.0)

    def gate_mm(ps, half, ti, stat, k, start, stop):
        """One N=512 matmul into ps[64*half : 64*half+64, :].

        The host packs every W k-chunk so sW[:, k, 2*half+ti, :] is the
        contiguous [gate0-half | gate1-half] 512-wide moving slice for
        PSUM tile ti (0=FI, 1=CO) and column group `half`.  Each matmul
        covers the tile's full FD width, so there is exactly one PSUM
        accumulation chain per (tile, partition-half).
        """
        # skip_group_check: CoreSim's zero-region tracking is
        # partition-blind; the two column groups' chains target
        # disjoint partition halves of the same bank, which is safe
        # under the per-element has_written HW semantics (the staged
        # baseline used the same pattern and measured correct on HW).
        nc.tensor.matmul(ps[64 * half:64 * half + 64, :],
                         stat, sW[:, k, 2 * half + ti, :],
                         start=start, stop=stop, skip_group_check=True)

    def emit_x_and_bias(t, pFI, pCO):
        """x-part + bias matmuls for step t (h-independent)."""
        xs = xpool.tile([128, KX, BC], BF16, tag="xs")
        nc.sync.dma_start(out=xs[:], in_=d_x[t])
        for ti, ps in ((0, pFI), (1, pCO)):
            for half in range(2):
                for kx in range(KX):
                    gate_mm(ps, half, ti, xs[:, kx, :], KH + kx,
                            kx == 0, False)
                gate_mm(ps, half, ti, s_ones[:], KH + KX,
                        False, t == 0)

    def emit_hpart(pFI, pCO, hT):
        """h-part matmuls, FI tile first so sigmoid(f/i) starts early.

        hT: [128, KH*BC] bf16; hT[:, 64j:64j+64] = stationary chunk j.
        """
        for ti, ps in ((0, pFI), (1, pCO)):
            for half in range(2):
                for j in range(KH):
                    gate_mm(ps, half, ti, hT[:, 64 * j:64 * j + 64], j,
                            False, j == KH - 1)

    # prologue: step 0 gates have no h contribution
    pFI = psFI.tile([128, 512], F32, tag="FI")
    pCO = psCO.tile([128, 512], F32, tag="CO")
    emit_x_and_bias(0, pFI, pCO)

    h = None
    for t in range(nsteps):
        last = t == nsteps - 1
        if not last:
            pFI_n = psFI.tile([128, 512], F32, tag="FI")
            pCO_n = psCO.tile([128, 512], F32, tag="CO")
            emit_x_and_bias(t + 1, pFI_n, pCO_n)

        # elementwise for step t, all [128, 256]
        sigf = gact.tile([128, HH], F32, tag="sigf")
        nc.scalar.activation(sigf[:], pFI[:, 0:256], AF.Sigmoid)
        sigi = gact.tile([128, HH], F32, tag="sigi")
        nc.scalar.activation(sigi[:], pFI[:, 256:512], AF.Sigmoid)
        tcb = gact.tile([128, HH], F32, tag="tcb")
        nc.scalar.activation(tcb[:], pCO[:, 0:256], AF.Tanh)
        sigo = gact.tile([128, HH], BF16, tag="sigo")
        nc.scalar.activation(sigo[:], pCO[:, 256:512], AF.Sigmoid)

        u1 = gact.tile([128, HH], F32, tag="u1")
        nc.vector.tensor_mul(u1[:], c_prev[:], sigf[:])
        u2 = gact.tile([128, HH], F32, tag="u2")
        nc.vector.tensor_mul(u2[:], tcb[:], sigi[:])
        c_new = state.tile([128, HH], F32, tag="c")
        nc.vector.tensor_add(c_new[:], u1[:], u2[:])
        tch = gact.tile([128, HH], BF16, tag="tch")
        nc.scalar.activation(tch[:], c_new[:], AF.Tanh)
        h = state.tile([128, HH], BF16, tag="h")
        nc.vector.tensor_mul(h[:], sigo[:], tch[:])
        c_prev = c_new

        if not last:
            # transpose h -> hT for the next step's stationary.  Block j
            # of h is [64, 128] at (part 64*(j//2), fd 128*(j%2)); PE
            # transpose requires base-partition-0 operands (base-64
            # inputs hang the exec unit), so stage h's upper half at
            # base 0 first.
            hhi = state.tile([BC, 2 * 128], BF16, tag="hhi")
            nc.vector.tensor_copy(hhi[:], h[64:128, :])
            pT = psT.tile([128, KH * BC], BF16, tag="hTp")
            for j in range(KH):
                blk = (h[0:64, 128 * j:128 * j + 128] if j < 2
                       else hhi[:, 128 * (j - 2):128 * (j - 2) + 128])
                nc.tensor.transpose(pT[:, 64 * j:64 * j + 64], blk,
                                    s_eye[0:64, :])
            hT = state.tile([128, KH * BC], BF16, tag="hT")
            nc.vector.tensor_copy(hT[:], pT[:])
            emit_hpart(pFI_n, pCO_n, hT)
            pFI, pCO = pFI_n, pCO_n

    # fc head: out = h @ fc_w.T + fc_b with h in [128, 256] layout;
    # batch b's result = r[b] + r[b + 64].
    m = gact.tile([128, HH], F32, tag="fcm")
    nc.vector.tensor_mul(m[:], h[:], s_fcw[:])
    r = gact.tile([128, 1], F32, tag="fcr")
    nc.vector.tensor_reduce(r[:], m[:], axis=mybir.AxisListType.X,
                            op=mybir.AluOpType.add)
    # two SBUF operands of a DVE op must share a base partition, so
    # stage the upper half at base 0 first
    rhi = gact.tile([BC, 1], F32, tag="fchi")
    nc.vector.tensor_copy(rhi[:], r[64:128, :])
    rs = gact.tile([BC, 1], F32, tag="fcs")
    nc.vector.tensor_add(rs[:], r[0:64, :], rhi[:])
    ro = gact.tile([BC, 1], F32, tag="fco")
    nc.vector.tensor_add(ro[:], rs[:], s_fcb[:])
    nc.sync.dma_start(out=d_out, in_=ro[:])


def _prep_core_inputs(x, W_w, W_b, fc_w, fc_b, core, nsteps=T):
    """Host-side shard + relayout for one core."""
    xs = x[core * BC:(core + 1) * BC, :nsteps]          # [BC, t, I]
    xt = np.ascontiguousarray(xs.transpose(1, 2, 0))    # [t, I, BC]
    xt = xt.reshape(nsteps, KX, 128, BC).transpose(0, 2, 1, 3)
    xt = np.ascontiguousarray(xt)                       # [t, 128, KX, BC]

    # W layout: [NK, 128, 4, H]; k-chunks 0..3 = Wh.T, 4..5 = Wx.T,
    # 6 = bias row.  Every k-chunk is packed in (tile, half) quadrants
    # q = 2*half + ti along dim 2:
    #   q=0: [f-lo | i-lo]   q=1: [chat-lo | o-lo]
    #   q=2: [f-hi | i-hi]   q=3: [chat-hi | o-hi]
    wfull = W_w.T.reshape(H + I, 4, 2, HH)          # [768, gate, half, 256]
    wb = W_b.reshape(4, 2, HH)                      # [gate, half, 256]
    wt = np.zeros((NK * 128, 4, H), dtype=np.float32)
    for half in range(2):
        for ti, gpair in enumerate((G_FI, G_CO)):
            q = 2 * half + ti
            wt[:H + I, q, 0:HH] = wfull[:, gpair[0], half]
            wt[:H + I, q, HH:H] = wfull[:, gpair[1], half]
            wt[H + I, q, 0:HH] = wb[gpair[0], half]
            wt[H + I, q, HH:H] = wb[gpair[1], half]
    wt = np.ascontiguousarray(wt.reshape(NK, 128, 4, H))

    ones_row = np.zeros((128, BC), dtype=np.float32)
    ones_row[0, :] = 1.0
    eye = np.concatenate([np.eye(BC, dtype=np.float32)] * 2, axis=0)
    # fc_w packed to the [128, 256] h layout: parts 0:64 = hid-lo,
    # 64:128 = hid-hi (same for every batch row in the 64-block).
    fcw = np.empty((128, HH), dtype=np.float32)
    fcw[0:64, :] = fc_w[0, 0:HH]
    fcw[64:128, :] = fc_w[0, HH:H]
    fcb = np.full((BC, 1), np.float32(fc_b[0]), dtype=np.float32)

    import ml_dtypes
    bf = ml_dtypes.bfloat16
    return {"xT": xt.astype(bf), "W": wt.astype(bf),
            "ones_row": ones_row.astype(bf), "eye": eye.astype(bf),
            "fcw": fcw, "fcb": fcb}


def kernel(x, W_w, W_b, fc_w, fc_b):
    x = np.asarray(x, dtype=np.float32)
    W_w = np.asarray(W_w, dtype=np.float32)
    W_b = np.asarray(W_b, dtype=np.float32)
    fc_w = np.asarray(fc_w, dtype=np.float32)
    fc_b = np.asarray(fc_b, dtype=np.float32)

    nc = _build(T)
    in_maps = [_prep_core_inputs(x, W_w, W_b, fc_w, fc_b, c)
               for c in range(NCORES)]
    res = run_bass_kernel_spmd(nc, in_maps, list(range(NCORES))).results
    return np.concatenate([res[c]["out"] for c in range(NCORES)], axis=0)
